# revision 12
# baseline (speedup 1.0000x reference)
"""Multi-head attention + residual + LayerNorm Trainium2 kernel (8-core SPMD).

The reference uses a *raw contiguous* head split: q/k/v [4096, 400] are
reinterpreted as [4, 4096, 100] ("Q16" = q.reshape(16384, 100) cut into 4
contiguous blocks).  Head h's 4096 "rows" are (token n = 1024h + m//4,
feature chunk j = m%4); row index inside the head block is 4*m_local + j.

Sharding (sequence parallel): core p owns attention rows [512p, 512(p+1)) of
every head, i.e. original tokens {1024h + 128p .. 1024h + 128(p+1)} per head
(512 tokens total, h-major order).  k/v and weights are replicated.

Device-internal orderings:
  - q-side rows are processed "j-outer" (rq'' = jf*128 + n_local); attention
    HBM writes and the output projection un-permute via strided APs.
  - the ctx path uses j-outer key ordering (a permutation of the softmax
    axis, exactly invariant) so v tiles stay natural feature slices.
Bias handling: bq is added at q-projection eviction (per-partition ACT bias);
bk/bv are added via a rank-1 ones-row matmul into the same PSUM accumulation.
Softmax skips the row-max subtraction (logits are O(10), exactly invariant).
Scores matmuls run in float32r; the ctx matmul runs in bf16 (error lands on a
term small against the residual); attention probabilities are fp32.
"""

import re
import sys
from contextlib import ExitStack

import numpy as np

sys.path.insert(0, "/opt/trn_rl_repo")

import concourse.bass as bass  # noqa: E402
import concourse.mybir as mybir  # noqa: E402
import bass_rust  # noqa: E402
from concourse import tile  # noqa: E402
from concourse.bass_utils import run_bass_kernel_spmd  # noqa: E402
from concourse.masks import make_identity  # noqa: E402
from concourse.vector_clock import ScopedClock  # noqa: E402

F32 = mybir.dt.float32
F32R = mybir.dt.float32r
BF16 = mybir.dt.bfloat16
AF = mybir.ActivationFunctionType
ALU = mybir.AluOpType
AX = mybir.AxisListType

N = 4096
D = 400
H = 4
DH = 100  # dim per head
NCORES = 8
NS = 512  # attention rows per core (per head)
TOK = 128  # original tokens per (core, head)
J = 4  # feature chunks of 100
M8 = 8  # 512-wide column chunks per head
SCALE = (DH // H) ** (-0.5)  # 0.2, the reference's quirky scale
EPS = 1e-5


def _drain_and_barrier_split(self, tick_clock, wait_clock):
    # walrus in this container rejects >1 sem wait per CTRL instruction; split
    # the TileContext final-drain waits across single-wait SP nops.
    nc = self.nc
    gc = tick_clock.global_clock
    vals = [int(x) for x in re.findall(r"\d+", repr(gc))]
    for i, v in enumerate(vals):
        if v == 0:
            continue
        part = [0] * len(vals)
        part[i] = v
        nop_inst = nc.sync.nop(nofuse=True)
        wait_clock.add_sem_waits(
            nop_inst.ins, ScopedClock({None: bass_rust.VectorClock(part)})
        )
    nc.sync.drain()
    nc.all_engine_barrier()
    assert self.sems is not None
    popped = nc._tile_sem_poison_stack.pop()
    assert popped is self._sem_poison
    sems = list(self.sems.allocated().values())
    for i in range(0, len(sems), 8):
        nc.clear_and_free_semaphores(sems[i : i + 8])
    nc.all_engine_barrier()


tile.TileContext._drain_and_barrier = _drain_and_barrier_split

_waitsplit_n = 0


def split_multi_waits(nc):
    """walrus here accepts at most one sem wait per instruction: hoist extra
    waits onto same-engine NoOps inserted immediately before the instruction."""
    global _waitsplit_n
    for func in nc.m.functions:
        for bb in func.blocks:
            new_insts = []
            for inst in bb.instructions:
                si = inst.sync_info
                if si is not None and si.on_wait and len(si.on_wait) > 1:
                    for w in si.on_wait[:-1]:
                        _waitsplit_n += 1
                        nop = mybir.InstNoOp(name=f"I-wsplit-{_waitsplit_n}")
                        nop.engine = inst.engine
                        nop.sync_info = mybir.SyncInfo(on_wait=[w], on_update=[])
                        new_insts.append(nop)
                    inst.sync_info = mybir.SyncInfo(
                        on_wait=[si.on_wait[-1]], on_update=si.on_update
                    )
                new_insts.append(inst)
            bb.instructions[:] = new_insts


def build_nc():
    nc = bass.Bass("TRN2", target_bir_lowering=False, debug=False)

    qT_in = nc.declare_dram_parameter("qT_in", [D, NS], F32R, isOutput=False)
    res_in = nc.declare_dram_parameter("res_in", [NS, D], F32, isOutput=False)
    keyT_in = nc.declare_dram_parameter("keyT_in", [D, N], F32R, isOutput=False)
    valT_in = nc.declare_dram_parameter("valT_in", [D, N], F32R, isOutput=False)
    WqT_in = nc.declare_dram_parameter("WqT_in", [D, D], F32R, isOutput=False)
    WkT_in = nc.declare_dram_parameter("WkT_in", [D, D], F32R, isOutput=False)
    WvT_in = nc.declare_dram_parameter("WvT_in", [D, D], F32R, isOutput=False)
    WoT_in = nc.declare_dram_parameter("WoT_in", [D, D], F32R, isOutput=False)
    bqT_in = nc.declare_dram_parameter("bqT_in", [DH, J], F32, isOutput=False)
    bk_in = nc.declare_dram_parameter("bk_in", [1, D], F32R, isOutput=False)
    ones_in = nc.declare_dram_parameter("ones_in", [1, NS], F32R, isOutput=False)
    bv_in = nc.declare_dram_parameter("bv_in", [1, D], F32R, isOutput=False)
    attn_out = nc.declare_dram_parameter("attn_out", [H, NS, N], F32, isOutput=True)
    out_out = nc.declare_dram_parameter("out_out", [NS, D], F32, isOutput=True)

    # view for interleaved attention-row writes: slab row 4*n + jf
    attn_v = attn_out.rearrange("h (n f) m -> h n f m", f=4)

    with tile.TileContext(nc) as tc, ExitStack() as ctx:
        # ---- persistent pools ----
        persist = ctx.enter_context(tc.tile_pool(name="persist", bufs=1))
        # [dd, j, token]: kT[j*100+dd, token]
        kT_sb = persist.tile([DH, J, N], F32R, name="kT_sb")
        qT_sb = persist.tile([DH, J, NS], F32R, name="qT_sb")
        v_sb = persist.tile([128, N // 128, D], BF16, name="v_sb")
        WoT_sb = [
            persist.tile([DH, D], F32R, name=f"WoT_sb{j}", tag=f"WoT_sb{j}")
            for j in range(J)
        ]
        res_sb = persist.tile([128, H, D], F32, name="res_sb")
        eps_t = persist.tile([128, 1], F32, name="eps_t")
        nc.vector.memset(eps_t[:], EPS)
        ones_sb = persist.tile([1, NS], F32R, name="ones_sb")
        nc.sync.dma_start(out=ones_sb[:], in_=ones_in[:])
        bk_sb = persist.tile([1, D], F32R, name="bk_sb")
        bv_sb = persist.tile([1, D], F32R, name="bv_sb")
        nc.sync.dma_start(out=bk_sb[:], in_=bk_in[:])
        nc.sync.dma_start(out=bv_sb[:], in_=bv_in[:])
        ident = persist.tile([128, 128], F32, name="ident")
        make_identity(nc, ident)
        expT_sb = persist.tile([128, N // 128, NS], BF16, name="expT_sb")

        pp_proj = ctx.enter_context(tc.tile_pool(name="pp_proj", bufs=2, space="PSUM"))

        # ---- prologue: load weights, project q, k, v ----
        with ExitStack() as pctx:
            wpool = pctx.enter_context(tc.tile_pool(name="wpool", bufs=1))
            WqT_sb = [
                wpool.tile([DH, D], F32R, name=f"WqT_sb{j}", tag=f"WqT_sb{j}")
                for j in range(J)
            ]
            WkT_sb = [
                wpool.tile([DH, D], F32R, name=f"WkT_sb{j}", tag=f"WkT_sb{j}")
                for j in range(J)
            ]
            WvT_sb = [
                wpool.tile([DH, D], F32R, name=f"WvT_sb{j}", tag=f"WvT_sb{j}")
                for j in range(J)
            ]
            bqT_sb = wpool.tile([DH, J], F32, name="bqT_sb")
            qTin_sb = wpool.tile([DH, J, NS], F32R, name="qTin_sb")
            kst_pool = pctx.enter_context(tc.tile_pool(name="kst", bufs=6))
            vst_pool = pctx.enter_context(tc.tile_pool(name="vst", bufs=6))

            for j in range(J):
                nc.sync.dma_start(out=WqT_sb[j][:], in_=WqT_in[j * DH : (j + 1) * DH, :])
                nc.sync.dma_start(out=WkT_sb[j][:], in_=WkT_in[j * DH : (j + 1) * DH, :])
                nc.sync.dma_start(out=WvT_sb[j][:], in_=WvT_in[j * DH : (j + 1) * DH, :])
                nc.sync.dma_start(
                    out=qTin_sb[:, j, :], in_=qT_in[j * DH : (j + 1) * DH, :]
                )
                nc.sync.dma_start(out=WoT_sb[j][:], in_=WoT_in[j * DH : (j + 1) * DH, :])
            nc.sync.dma_start(out=bqT_sb[:], in_=bqT_in[:])
            for h in range(H):
                nc.sync.dma_start(
                    out=res_sb[:, h, :], in_=res_in[h * TOK : (h + 1) * TOK, :]
                )

            # q projection: qT[jo*100+dd, n] = sum_j WqT[j.., jo..] qTin[j.., n] + bq
            for jo in range(J):
                ps_q = pp_proj.tile([DH, NS], F32, name="ps_q", tag="proj")
                for j in range(J):
                    nc.tensor.matmul(
                        ps_q[:],
                        WqT_sb[j][:, jo * DH : (jo + 1) * DH],
                        qTin_sb[:, j, :],
                        start=(j == 0),
                        stop=(j == J - 1),
                    )
                nc.scalar.activation(
                    qT_sb[:, jo, :],
                    ps_q[:],
                    AF.Identity,
                    bias=bqT_sb[:, jo : jo + 1],
                    scale=1.0,
                )

            # k projection (+bk via ones-row rank-1 matmul)
            for m8 in range(M8):
                ksts = []
                for j in range(J):
                    kst = kst_pool.tile([DH, 512], F32R, name="kst")
                    nc.sync.dma_start(
                        out=kst[:],
                        in_=keyT_in[j * DH : (j + 1) * DH, m8 * 512 : (m8 + 1) * 512],
                    )
                    ksts.append(kst)
                for jo in range(J):
                    ps_k = pp_proj.tile([DH, 512], F32, name="ps_k", tag="proj")
                    for j in range(J):
                        nc.tensor.matmul(
                            ps_k[:],
                            WkT_sb[j][:, jo * DH : (jo + 1) * DH],
                            ksts[j][:],
                            start=(j == 0),
                            stop=False,
                        )
                    nc.tensor.matmul(
                        ps_k[:],
                        bk_sb[:, jo * DH : (jo + 1) * DH],
                        ones_sb[:, :512],
                        start=False,
                        stop=True,
                    )
                    nc.vector.tensor_copy(
                        kT_sb[:, jo, m8 * 512 : (m8 + 1) * 512], ps_k[:]
                    )

            # v projection (natural [token, 400] layout, bf16, +bv via ones-row)
            for m8 in range(M8):
                vsts = []
                for j in range(J):
                    vst = vst_pool.tile([DH, 512], F32R, name="vst")
                    nc.sync.dma_start(
                        out=vst[:],
                        in_=valT_in[j * DH : (j + 1) * DH, m8 * 512 : (m8 + 1) * 512],
                    )
                    vsts.append(vst)
                for mi in range(4):
                    ps_v = pp_proj.tile([128, D], F32, name="ps_v", tag="proj")
                    for j in range(J):
                        nc.tensor.matmul(
                            ps_v[:],
                            vsts[j][:, mi * 128 : (mi + 1) * 128],
                            WvT_sb[j][:],
                            start=(j == 0),
                            stop=False,
                        )
                    nc.tensor.matmul(
                        ps_v[:],
                        ones_sb[:, :128],
                        bv_sb[:],
                        start=False,
                        stop=True,
                    )
                    nc.vector.tensor_copy(v_sb[:, m8 * 4 + mi, :], ps_v[:])

        # ---- main pools ----
        attn_pool = ctx.enter_context(tc.tile_pool(name="attn", bufs=2))
        ctx_pool = ctx.enter_context(tc.tile_pool(name="ctxp", bufs=2))
        small = ctx.enter_context(tc.tile_pool(name="small", bufs=10))
        rsum_pool = ctx.enter_context(tc.tile_pool(name="rsum", bufs=10))
        ln_pool = ctx.enter_context(tc.tile_pool(name="ln", bufs=2))
        pp_o1 = ctx.enter_context(tc.tile_pool(name="pp_o1", bufs=2, space="PSUM"))
        pp_o2 = ctx.enter_context(tc.tile_pool(name="pp_o2", bufs=2, space="PSUM"))
        pp_ctx = ctx.enter_context(tc.tile_pool(name="pp_ctx", bufs=1, space="PSUM"))
        pp_op = ctx.enter_context(tc.tile_pool(name="pp_op", bufs=1, space="PSUM"))

        for h in range(H):
            hcol = h * (N // 4)  # head h's token-column base in kT/v
            # --- orientation 1: scores rows -> softmax + fp32 attn output ---
            rsums = []
            for jf in range(J):
                attn_t = attn_pool.tile([128, N], F32, name="attn_t")
                accum8 = small.tile([128, M8], F32, name="accum8")
                lhs_q = qT_sb[:, jf, h * TOK : (h + 1) * TOK]
                for m8 in range(M8):
                    # ref column order 4m+j: token-inner-..., j fastest
                    rhs_k = kT_sb[
                        :, :, hcol + m8 * TOK : hcol + (m8 + 1) * TOK
                    ].rearrange("p a b -> p b a")
                    ps1 = pp_o1.tile([128, 512], F32, name="ps1")
                    nc.tensor.matmul(ps1[:], lhs_q, rhs_k, start=True, stop=True)
                    nc.scalar.activation(
                        attn_t[:, m8 * 512 : (m8 + 1) * 512],
                        ps1[:],
                        AF.Exp,
                        scale=SCALE,
                        accum_out=accum8[:, m8 : m8 + 1],
                    )
                ssum = small.tile([128, 1], F32, name="ssum")
                nc.vector.reduce_sum(ssum[:], accum8[:], axis=AX.X)
                rs = rsum_pool.tile([128, 1], F32, name="rs")
                nc.vector.reciprocal(rs[:], ssum[:])
                rsums.append(rs)
                nc.vector.tensor_scalar_mul(attn_t[:], attn_t[:], rs[:])
                nc.sync.dma_start(out=attn_v[h, :, jf, :], in_=attn_t[:])

            # --- orientation 2: scoresT, j-outer key rows, unnormalized bf16 ---
            rhs_q = qT_sb[:, :, h * TOK : (h + 1) * TOK]  # [100, 4, 128] = rq''
            for j in range(J):
                for tl in range(M8):
                    idx = j * M8 + tl
                    ps2 = pp_o2.tile([128, NS], F32, name="ps2")
                    nc.tensor.matmul(
                        ps2[:],
                        kT_sb[:, j, hcol + tl * 128 : hcol + (tl + 1) * 128],
                        rhs_q,
                        start=True,
                        stop=True,
                    )
                    nc.scalar.activation(
                        expT_sb[:, idx, :], ps2[:], AF.Exp, scale=SCALE
                    )

            # --- ctx^T (unnormalized): [dd, rq''] over j-outer key rows ---
            ps_c = pp_ctx.tile([DH, NS], F32, name="ps_c")
            for j in range(J):
                for tl in range(M8):
                    idx = j * M8 + tl
                    nc.tensor.matmul(
                        ps_c[:],
                        v_sb[:, h * M8 + tl, j * DH : (j + 1) * DH],
                        expT_sb[:, idx, :],
                        start=(idx == 0),
                        stop=(idx == N // 128 - 1),
                    )
            ctxT_u = ctx_pool.tile([DH, NS], F32, name="ctxT_u", tag="ctxT_u")
            nc.scalar.copy(ctxT_u[:], ps_c[:])

            # --- normalize columns via double transpose (tiny tiles) ---
            ctxn = ctx_pool.tile([128, J, DH], F32, name="ctxn", tag="ctxn")
            ctxnT = ctx_pool.tile([DH, J, 128], F32R, name="ctxnT", tag="ctxnT")
            for jf in range(J):
                psT1 = pp_proj.tile([128, DH], F32, name="psT1", tag="proj")
                nc.tensor.transpose(
                    psT1[:], ctxT_u[:, jf * 128 : (jf + 1) * 128], ident[:DH, :DH]
                )
                nc.vector.tensor_scalar_mul(ctxn[:, jf, :], psT1[:], rsums[jf][:])
                psT2 = pp_proj.tile([DH, 128], F32, name="psT2", tag="proj")
                nc.tensor.transpose(psT2[:], ctxn[:, jf, :], ident[:])
                nc.scalar.copy(ctxnT[:, jf, :], psT2[:])

            # --- output projection + residual + LayerNorm for this head ---
            ps_o = pp_op.tile([128, D], F32, name="ps_o")
            for jf in range(J):
                nc.tensor.matmul(
                    ps_o[:],
                    ctxnT[:, jf, :],
                    WoT_sb[jf][:],
                    start=(jf == 0),
                    stop=(jf == J - 1),
                )
            x_sb = ln_pool.tile([128, D], F32, name="x_sb")
            xsum = small.tile([128, 1], F32, name="xsum")
            nc.vector.scalar_tensor_tensor(
                x_sb[:],
                ps_o[:],
                1.0,
                res_sb[:, h, :],
                op0=ALU.mult,
                op1=ALU.add,
                accum_out=xsum[:],
            )
            sq = ln_pool.tile([128, D], F32, name="sq")
            sumsq = small.tile([128, 1], F32, name="sumsq")
            nc.scalar.activation(sq[:], x_sb[:], AF.Square, accum_out=sumsq[:])
            mean = small.tile([128, 1], F32, name="mean")
            nc.vector.tensor_scalar_mul(mean[:], xsum[:], 1.0 / D)
            meansq = small.tile([128, 1], F32, name="meansq")
            nc.vector.tensor_mul(meansq[:], mean[:], mean[:])
            var_t = small.tile([128, 1], F32, name="var_t")
            nc.vector.scalar_tensor_tensor(
                var_t[:], sumsq[:], 1.0 / D, meansq[:], op0=ALU.mult, op1=ALU.subtract
            )
            sdev = small.tile([128, 1], F32, name="sdev")
            nc.scalar.activation(sdev[:], var_t[:], AF.Sqrt, bias=eps_t[:], scale=1.0)
            rstd = small.tile([128, 1], F32, name="rstd")
            nc.vector.reciprocal(rstd[:], sdev[:])
            out_t = ln_pool.tile([128, D], F32, name="out_t")
            nc.vector.tensor_scalar(
                out_t[:], x_sb[:], mean[:], rstd[:], op0=ALU.subtract, op1=ALU.mult
            )
            nc.sync.dma_start(
                out=out_out[h * TOK : (h + 1) * TOK, :], in_=out_t[:]
            )

    split_multi_waits(nc)
    return nc


_NC_CACHE = None


def _get_nc():
    global _NC_CACHE
    if _NC_CACHE is None:
        _NC_CACHE = build_nc()
    return _NC_CACHE


def core_tokens(p):
    return np.concatenate(
        [1024 * h + np.arange(TOK * p, TOK * (p + 1)) for h in range(H)]
    )


def host_prep(key, value, query, Wq, bq, Wk, bk, Wv, bv, Wo, bo):
    keyT = np.ascontiguousarray(key.T)
    valT = np.ascontiguousarray(value.T)
    queryT = np.ascontiguousarray(query.T)
    shared = {
        "keyT_in": keyT,
        "valT_in": valT,
        "WqT_in": np.ascontiguousarray(Wq.T),
        "WkT_in": np.ascontiguousarray(Wk.T),
        "WvT_in": np.ascontiguousarray(Wv.T),
        "WoT_in": np.ascontiguousarray(Wo.T),
        "bqT_in": np.ascontiguousarray(bq.reshape(J, DH).T),
        "bk_in": np.ascontiguousarray(bk.reshape(1, D)),
        "ones_in": np.ones((1, NS), np.float32),
        "bv_in": np.ascontiguousarray(bv.reshape(1, D)),
    }
    in_maps = []
    for p in range(NCORES):
        toks = core_tokens(p)
        in_maps.append(
            {
                "qT_in": np.ascontiguousarray(queryT[:, toks]),
                "res_in": np.ascontiguousarray(query[toks] + bo),
                **shared,
            }
        )
    return in_maps


def kernel(key, value, query, Wq, bq, Wk, bk, Wv, bv, Wo, bo, gamma, beta):
    f32 = lambda a: np.asarray(a, np.float32)
    key, value, query = f32(key), f32(value), f32(query)
    Wq, bq, Wk, bk = f32(Wq), f32(bq), f32(Wk), f32(bk)
    Wv, bv, Wo, bo = f32(Wv), f32(bv), f32(Wo), f32(bo)
    gamma, beta = f32(gamma), f32(beta)

    in_maps = host_prep(key, value, query, Wq, bq, Wk, bk, Wv, bv, Wo, bo)
    nc = _get_nc()
    res = run_bass_kernel_spmd(nc, in_maps, list(range(NCORES)))

    out = np.empty((N, D), np.float32)
    for p in range(NCORES):
        out[core_tokens(p)] = res.results[p]["out_out"]
    attn = np.concatenate(
        [res.results[p]["attn_out"] for p in range(NCORES)], axis=1
    )
    if not (np.all(gamma == 1.0) and np.all(beta == 0.0)):
        out = out * gamma + beta
    return out, attn
